# revision 30
# baseline (speedup 1.0000x reference)
"""Trainium2 Bass kernel for nn_Encoder (GNN message-passing encoder).

Self-contained: accepts FULL inputs, shards across 8 NeuronCores (data
parallel over particles/edges, params replicated), runs one SPMD Bass
program via run_bass_kernel_spmd, and gathers the full output.

Outputs match reference: (nodes [1,N,128] f32, edges [E,128] f32,
neighbor_idxs [E,3] i32 passthrough).

Pipeline (per core), all in fp32 with fp32r matmuls:
  edges: indirect-gather P[recv],P[send] -> rel/dist -> packed PE
  transpose to feature-major -> mm1(K=4, ones-row folds b1) -> relu ->
  mm2 -> relu(+b2) -> edge-major mm3 (lhsT=h2r tile, rhs=W3 zero-padded
  to 256 cols for full-rate fp32r) + fp16 ones x b3 bias matmul ->
  PSUM->SBUF evac -> batched bn_stats LayerNorm -> per-tile affine apply
  -> edge-major output DMA.  Nodes: same skeleton with K=31 features
  (vel, mat_W gather (mat_b folded into bias row on device), wall clips).
"""

import os
from contextlib import ExitStack

import numpy as np

import concourse.bacc as bacc
import concourse.bass as bass
import concourse.mybir as mybir
import concourse.tile as tile
from concourse import bass_utils
from concourse.bass import IndirectOffsetOnAxis
from concourse.masks import make_identity
from bass_rust import add_dep_helper

F32 = mybir.dt.float32
F32R = mybir.dt.float32r
F16 = mybir.dt.float16
I32 = mybir.dt.int32
AF = mybir.ActivationFunctionType
OP = mybir.AluOpType

P = 128
N_CORES = 8
E_TOT = 1_000_000
N_TOT = 50_000
HID = 128
R_CONN = 0.015
EPS = 1e-5

# ---- geometry (NSB_E env-overridable for small-scale simulator tests) ----
CH_E = 4                                    # transpose chunks / edge superblock
NSB_E = int(os.environ.get("K_NSB_E", 8))   # edge superblocks per core
CH_N = 16                                   # node chunks (of 512 nodes)

SB_E = CH_E * 32 * P                        # edges per superblock (16384)
E_PAD = NSB_E * SB_E                        # padded edges per core (131072)
SB_N = CH_N * 4 * P                         # padded nodes per core (8192)
N_PAD = SB_N

E_CORE = E_TOT // N_CORES                   # 125000
N_CORE = N_TOT // N_CORES                   # 6250

_PROGRAM_CACHE: dict = {}

def _ensure_ntff_hook():
    """Install the axon NTFF profiling hook if the glue module is absent.

    Only needed when BASS_TRACE=1; harmless otherwise."""
    try:
        import antenv.axon_hooks  # noqa: F401
        return
    except ImportError:
        pass
    import sys
    import types

    import antenv

    mod = types.ModuleType("antenv.axon_hooks")
    mod._hook = None

    def set_axon_ntff_profile_hook(h):
        mod._hook = h

    def get_axon_ntff_profile_hook():
        return mod._hook

    mod.set_axon_ntff_profile_hook = set_axon_ntff_profile_hook
    mod.get_axon_ntff_profile_hook = get_axon_ntff_profile_hook
    sys.modules["antenv.axon_hooks"] = mod
    antenv.axon_hooks = mod
    try:
        from trn_agent_boot.trn_boot import _ntff_profile_via_ctypes

        hook = _ntff_profile_via_ctypes("/opt/axon/libaxon_pjrt.so")
        if hook is not None:
            set_axon_ntff_profile_hook(hook)
    except Exception:
        pass




def _ln_combine(nc, scp, stats, nt, name, epst):
    """bn_stats even/odd 6-tuples [128,nt,6] -> (s, b) with y = s*x + b
    equivalent to (x - mean) / sqrt(var + EPS)."""
    me = stats[:, :, 1]
    mo = stats[:, :, 4]
    m2e = stats[:, :, 2]
    m2o = stats[:, :, 5]
    msum = scp.tile([P, nt], F32, tag=f"{name}_msum")
    nc.vector.tensor_tensor(out=msum[:], in0=me, in1=mo, op=OP.add)
    dlt = scp.tile([P, nt], F32, tag=f"{name}_dlt")
    nc.vector.tensor_tensor(out=dlt[:], in0=me, in1=mo, op=OP.subtract)
    dlt2 = scp.tile([P, nt], F32, tag=f"{name}_dlt2")
    nc.vector.tensor_tensor(out=dlt2[:], in0=dlt[:], in1=dlt[:], op=OP.mult)
    m2s = scp.tile([P, nt], F32, tag=f"{name}_m2s")
    nc.vector.tensor_tensor(out=m2s[:], in0=m2e, in1=m2o, op=OP.add)
    # combined M2 = m2e + m2o + (ne*no/n) * dlt^2 ; ne=no=64 -> 32
    dlt2b = scp.tile([P, nt], F32, tag=f"{name}_dlt2b")
    nc.vector.tensor_scalar(
        out=dlt2b[:], in0=dlt2[:], scalar1=float(HID // 4), scalar2=None, op0=OP.mult
    )
    m2 = scp.tile([P, nt], F32, tag=f"{name}_m2")
    nc.vector.tensor_tensor(out=m2[:], in0=m2s[:], in1=dlt2b[:], op=OP.add)
    sd = scp.tile([P, nt], F32, tag=f"{name}_sd")
    nc.scalar.activation(
        out=sd[:], in_=m2[:], func=AF.Sqrt, bias=epst[:], scale=1.0 / HID
    )
    s = scp.tile([P, nt], F32, tag=f"{name}_s")
    nc.vector.reciprocal(out=s[:], in_=sd[:])
    # b = -0.5 * msum * s
    nms = scp.tile([P, nt], F32, tag=f"{name}_nms")
    nc.vector.tensor_tensor(out=nms[:], in0=msum[:], in1=s[:], op=OP.mult)
    b = scp.tile([P, nt], F32, tag=f"{name}_b")
    nc.vector.tensor_scalar(
        out=b[:], in0=nms[:], scalar1=-0.5, scalar2=None, op0=OP.mult
    )
    return s, b


def _build_program(e_gb_trivial: bool, n_gb_trivial: bool):
    nc = bacc.Bacc("TRN2", target_bir_lowering=False, debug=False)

    # ----------------- DRAM tensors -----------------
    d_pr = nc.dram_tensor("pr_in", [NSB_E, P, CH_E * 32, 2], F32, kind="ExternalInput")
    d_ps = nc.dram_tensor("ps_in", [NSB_E, P, CH_E * 32, 2], F32, kind="ExternalInput")
    d_eW1 = nc.dram_tensor("eW1", [3, HID], F32, kind="ExternalInput")
    d_eb1 = nc.dram_tensor("eb1", [1, HID], F32, kind="ExternalInput")
    d_eW2 = nc.dram_tensor("eW2", [HID, HID], F32, kind="ExternalInput")
    d_eb2 = nc.dram_tensor("eb2", [HID, 1], F32, kind="ExternalInput")
    d_eW3 = nc.dram_tensor("eW3", [HID, HID], F32, kind="ExternalInput")
    d_eb3 = nc.dram_tensor("eb3", [1, HID], F32, kind="ExternalInput")
    d_eg = nc.dram_tensor("e_gamma", [HID], F32, kind="ExternalInput")
    d_ebt = nc.dram_tensor("e_beta", [HID], F32, kind="ExternalInput")

    d_velp = nc.dram_tensor("velp", [N_PAD, 10], F32, kind="ExternalInput")
    d_posp = nc.dram_tensor("posp", [N_PAD, 2], F32, kind="ExternalInput")
    d_memb = nc.dram_tensor("memb_in", [N_PAD, 16], F32, kind="ExternalInput")
    d_matb = nc.dram_tensor("mat_b", [16, 1], F32, kind="ExternalInput")
    d_nW1 = nc.dram_tensor("nW1", [30, HID], F32, kind="ExternalInput")
    d_nb1 = nc.dram_tensor("nb1", [1, HID], F32, kind="ExternalInput")
    d_nW2 = nc.dram_tensor("nW2", [HID, HID], F32, kind="ExternalInput")
    d_nb2 = nc.dram_tensor("nb2", [HID, 1], F32, kind="ExternalInput")
    d_nW3 = nc.dram_tensor("nW3", [HID, HID], F32, kind="ExternalInput")
    d_nb3 = nc.dram_tensor("nb3", [1, HID], F32, kind="ExternalInput")
    d_ng = nc.dram_tensor("n_gamma", [HID], F32, kind="ExternalInput")
    d_nbt = nc.dram_tensor("n_beta", [HID], F32, kind="ExternalInput")

    d_eout = nc.dram_tensor("edges_out", [E_PAD, HID], F32, kind="ExternalOutput")
    d_nout = nc.dram_tensor("nodes_out", [N_PAD, HID], F32, kind="ExternalOutput")

    with ExitStack() as ctx:
        tc = ctx.enter_context(tile.TileContext(nc))
        const = ctx.enter_context(tc.tile_pool(name="const", bufs=1))
        idxp = ctx.enter_context(tc.tile_pool(name="idxp", bufs=3))
        gath = ctx.enter_context(tc.tile_pool(name="gath", bufs=2))
        featp = ctx.enter_context(tc.tile_pool(name="featp", bufs=2))
        sqp = ctx.enter_context(tc.tile_pool(name="sqp", bufs=2))
        ftp = ctx.enter_context(tc.tile_pool(name="ftp", bufs=2))
        fbp = ctx.enter_context(tc.tile_pool(name="fbp", bufs=2))
        nodep = ctx.enter_context(tc.tile_pool(name="nodep", bufs=1))
        h1rp = ctx.enter_context(tc.tile_pool(name="h1rp", bufs=8))
        h2rp = ctx.enter_context(tc.tile_pool(name="h2rp", bufs=3))
        zstp = ctx.enter_context(tc.tile_pool(name="zstp", bufs=3))
        st2p = ctx.enter_context(tc.tile_pool(name="st2p", bufs=2))
        statp = ctx.enter_context(tc.tile_pool(name="statp", bufs=3))
        scp = ctx.enter_context(tc.tile_pool(name="scp", bufs=3))

        tpp = ctx.enter_context(tc.tile_pool(name="tpp", bufs=2, space="PSUM"))
        h12p = ctx.enter_context(tc.tile_pool(name="h12p", bufs=2, space="PSUM"))
        m3pp = ctx.enter_context(tc.tile_pool(name="m3pp", bufs=2, space="PSUM"))

        # ----------------- constants -----------------
        ident = const.tile([P, P], F32)
        make_identity(nc, ident[:])

        ones16 = const.tile([1, HID], F16)
        nc.vector.memset(ones16[:], 1.0)

        epst = const.tile([P, 1], F32)
        nc.vector.memset(epst[:], EPS)

        def load_w(name, dram, shape, dtype=F32):
            t = const.tile(list(shape), dtype, tag=name)
            if dtype == F16:
                nc.gpsimd.dma_start(out=t[:], in_=dram.ap())  # SWDGE casts
            else:
                nc.sync.dma_start(out=t[:], in_=dram.ap())
            return t

        eW1aug = const.tile([4, HID], F16)
        nc.gpsimd.dma_start(out=eW1aug[0:3, :], in_=d_eW1.ap())
        nc.gpsimd.dma_start(out=eW1aug[3:4, :], in_=d_eb1.ap())
        eW2 = load_w("eW2", d_eW2, [HID, HID], F16)
        eb2 = load_w("eb2", d_eb2, [HID, 1])
        eW3_16 = load_w("eW3_16", d_eW3, [HID, HID], F16)
        # b3 broadcast tile (added during PSUM evac)
        eb3bc = const.tile([P, HID], F32, tag="eb3bc")
        nc.sync.dma_start(
            out=eb3bc[:], in_=bass.AP(tensor=d_eb3, offset=0, ap=[[0, P], [1, HID]])
        )

        nW2 = load_w("nW2", d_nW2, [HID, HID], F16)
        nb2 = load_w("nb2", d_nb2, [HID, 1])
        nW3_16 = load_w("nW3_16", d_nW3, [HID, HID], F16)
        nb3bc = const.tile([P, HID], F32, tag="nb3bc")
        nc.sync.dma_start(
            out=nb3bc[:], in_=bass.AP(tensor=d_nb3, offset=0, ap=[[0, P], [1, HID]])
        )

        matb = load_w("matb", d_matb, [16, 1])

        # nW1aug: rows 0..29 = nW1, row 30 = nb1 + mat_b @ nW1[10:26], row 31 = 0
        nW1aug = const.tile([32, HID], F16)
        nc.gpsimd.dma_start(out=nW1aug[0:30, :], in_=d_nW1.ap())
        nb1tmp = load_w("nb1tmp", d_nb1, [1, HID])
        # base-partition-0 copy of nW1 rows 10..25 (matmul rhs must start at 0)
        nW1mid = const.tile([16, HID], F32)
        nc.sync.dma_start(out=nW1mid[:], in_=d_nW1.ap()[10:26, :])
        nb1x = tpp.tile([1, HID], F32, tag="tp")
        nc.tensor.matmul(
            out=nb1x[:], lhsT=matb[:], rhs=nW1mid[:], start=True, stop=True
        )
        nb1row = const.tile([1, HID], F16)
        nc.vector.tensor_tensor(
            out=nb1row[:], in0=nb1tmp[:], in1=nb1x[:], op=OP.add
        )
        nc.sync.dma_start(out=nW1aug[30:31, :], in_=nb1row[:])

        def gb_tiles(d_g, d_b, name):
            g_bc = const.tile([P, HID], F32, tag=f"{name}_gbc")
            b_bc = const.tile([P, HID], F32, tag=f"{name}_bbc")
            nc.sync.dma_start(
                out=g_bc[:], in_=bass.AP(tensor=d_g, offset=0, ap=[[0, P], [1, HID]])
            )
            nc.sync.dma_start(
                out=b_bc[:], in_=bass.AP(tensor=d_b, offset=0, ap=[[0, P], [1, HID]])
            )
            return g_bc, b_bc

        e_gbc = None if e_gb_trivial else gb_tiles(d_eg, d_ebt, "e")
        n_gbc = None if n_gb_trivial else gb_tiles(d_ng, d_nbt, "n")

        def bc16(t):  # [P, HID] tile -> [P, 16, HID] free-broadcast AP
            a = t[:]
            return bass.AP(tensor=a.tensor, offset=a.offset,
                           ap=[a.ap[0], [0, 16], a.ap[1]])

        # ---------------------------------------------------------
        # shared tail: mm2 -> relu2 -> mm3(+bias) -> evac -> LN -> out
        # ---------------------------------------------------------
        def mlp_tail(pfx, n_sb, sb_tiles, W2, b2, W3_16, b3bc, gbc,
                     out_dram_r, h1r_of):
            ngrp = sb_tiles // 16
            for sb in range(n_sb):
                for grp in range(ngrp):
                    zst = zstp.tile([P, 8, 2 * HID], F32, tag="zst")
                    stats = statp.tile([P, 8, 8], F32, tag=f"{pfx}_stats")
                    for half in range(2):
                        zp = m3pp.tile([P, 8, HID], F32, tag="zp")
                        for sub, jj in enumerate(
                            (grp * 4 + half * 2, grp * 4 + half * 2 + 1)
                        ):
                            h1r = h1r_of(sb, jj)
                            h2ps = h12p.tile([P, 512], F32, tag="h12")
                            nc.tensor.matmul(
                                out=h2ps[:],
                                lhsT=W2[:],
                                rhs=h1r[:],
                                start=True, stop=True,
                            )
                            h2r = h2rp.tile([P, 512], F16, tag="h2r")
                            nc.scalar.activation(
                                out=h2r[:], in_=h2ps[:], func=AF.Relu,
                                bias=b2[:], scale=1.0,
                            )
                            for tl in range(4):
                                nc.tensor.matmul(
                                    out=zp[:, sub * 4 + tl, :],
                                    lhsT=h2r[:, tl * P:(tl + 1) * P],
                                    rhs=W3_16[:],
                                    start=True, stop=True,
                                )
                        b3v = b3bc[:]
                        z = zst[:, half * 4:(half + 1) * 4, :]
                        zi = bass.AP(
                            tensor=z.tensor, offset=z.offset,
                            ap=[z.ap[0], z.ap[1], [1, 2], [2, HID]],
                        )
                        nc.vector.tensor_tensor(
                            out=zi,
                            in0=zp[:, :, :].rearrange("p (pr tl) f -> p pr tl f", tl=2),
                            in1=bass.AP(tensor=b3v.tensor, offset=b3v.offset,
                                        ap=[b3v.ap[0], [0, 4], [0, 2], b3v.ap[1]]),
                            op=OP.add,
                        )
                    for pr in range(8):
                        nc.vector.bn_stats(
                            out=stats[:, pr, 0:6],
                            in_=zst[:, pr, :],
                        )
                    # per-tile mean/var live directly in even/odd slots
                    m_v = stats[:, :, 1:5:3]       # [P, 8, 2] means
                    var_v = stats[:, :, 2:6:3]     # [P, 8, 2] M2 (count=128)
                    sd = scp.tile([P, 8, 2], F32, tag=f"{pfx}_sd")
                    nc.scalar.activation(
                        out=sd[:], in_=var_v, func=AF.Sqrt, bias=epst[:],
                        scale=1.0 / HID,
                    )
                    s_t = scp.tile([P, 8, 2], F32, tag=f"{pfx}_s")
                    nc.vector.reciprocal(out=s_t[:], in_=sd[:])
                    nms = scp.tile([P, 8, 2], F32, tag=f"{pfx}_nms")
                    nc.vector.tensor_tensor(out=nms[:], in0=m_v, in1=s_t[:], op=OP.mult)
                    b_t = scp.tile([P, 8, 2], F32, tag=f"{pfx}_b")
                    nc.vector.tensor_scalar(
                        out=b_t[:], in0=nms[:], scalar1=-1.0, scalar2=None, op0=OP.mult
                    )
                    st2 = st2p.tile([P, 16, HID], F32, tag="st2")
                    for tl in range(16):
                        pr, par = tl // 2, tl % 2
                        sc = s_t[:, pr, par:par + 1]
                        bi = b_t[:, pr, par:par + 1]
                        zin = zst[:, pr, par:par + 2 * HID - 1:2]
                        if tl % 4 == 0:
                            nc.scalar.activation(
                                out=st2[:, tl, :], in_=zin,
                                func=AF.Identity, bias=bi, scale=sc,
                            )
                        else:
                            nc.gpsimd.tensor_scalar(
                                out=st2[:, tl, :], in0=zin,
                                scalar1=sc, scalar2=bi, op0=OP.mult, op1=OP.add,
                            )
                    if gbc is not None:
                        g_bc, b_bc = gbc
                        nc.vector.tensor_tensor(
                            out=st2[:], in0=st2[:], in1=bc16(g_bc), op=OP.mult
                        )
                        nc.vector.tensor_tensor(
                            out=st2[:], in0=st2[:], in1=bc16(b_bc), op=OP.add
                        )
                    nc.sync.dma_start(out=out_dram_r[sb, grp], in_=st2[:])

        # ---------------------------------------------------------
        # EDGE branch
        # ---------------------------------------------------------
        e_ngrp = (CH_E * 32) // 16  # staging groups per edge superblock
        eout_r = d_eout.ap().rearrange(
            "(sb grp tl p) f -> sb grp p tl f", sb=NSB_E, grp=e_ngrp, tl=16, p=P
        )

        edge_h1r: dict = {}

        def edge_mm1(sb):
            pr = gath.tile([P, CH_E * 32, 2], F32, tag="pr")
            nc.sync.dma_start(out=pr[:], in_=d_pr.ap()[sb])
            ps = gath.tile([P, CH_E * 32, 2], F32, tag="ps")
            nc.sync.dma_start(out=ps[:], in_=d_ps.ap()[sb])
            # f-major packed layout: feat[p, c, f, g] -> transpose puts
            # feature f of group g at partition 32f+g (contiguous f bands)
            feat = featp.tile([P, CH_E, 4, 32], F32, tag="efeat")
            prv = pr[:].rearrange("p (c g) x -> p c x g", c=CH_E)
            psv = ps[:].rearrange("p (c g) x -> p c x g", c=CH_E)
            nc.gpsimd.tensor_tensor(
                out=feat[:, :, 0:2, :], in0=prv, in1=psv, op=OP.subtract
            )
            sq = sqp.tile([P, CH_E, 2, 32], F32, tag="esq")
            nc.gpsimd.tensor_tensor(
                out=sq[:], in0=feat[:, :, 0:2, :], in1=feat[:, :, 0:2, :], op=OP.mult
            )
            d2 = sqp.tile([P, CH_E, 32], F32, tag="ed2")
            nc.gpsimd.tensor_tensor(
                out=d2[:], in0=sq[:, :, 0, :], in1=sq[:, :, 1, :], op=OP.add
            )
            nc.scalar.activation(out=feat[:, :, 2, :], in_=d2[:], func=AF.Sqrt)
            nc.vector.memset(feat[:, :, 3, :], 1.0)

            featT = ftp.tile([P, CH_E, P], F16, tag="efeatT")
            for c in range(CH_E):
                tp = tpp.tile([P, P], F32, tag="tp")
                nc.tensor.transpose(out=tp[:], in_=feat[:, c, :, :], identity=ident[:])
                nc.vector.tensor_copy(out=featT[:, c, :], in_=tp[:])

            return featT

        def edge_mm1_block(featT, blk):
            # repack so each group's 4 feature rows start at partition 0
            # (matmul operands must have base partition 0/32/64)
            featB = fbp.tile([4, 8, CH_E * P], F16, tag="efeatB")
            for f in range(4):
                s = 32 * f + 8 * blk
                nc.sync.dma_start(
                    out=featB[f:f + 1, :, :],
                    in_=featT[s:s + 8, :, :],
                )
            # join op: single wait target covering all 4 repack DMAs so the
            # consuming matmuls stay under the HW sync-wait-command limit
            jt = scp.tile([4, 1], F16, tag="ejoin")
            join = nc.vector.tensor_copy(out=jt[:], in_=featB[:, 0, 0:1])
            tiles = {}
            for gl in range(8):
                g = blk * 8 + gl
                h1ps = h12p.tile([P, 512], F32, tag="h12")
                mm = nc.tensor.matmul(
                    out=h1ps[:],
                    lhsT=eW1aug[:],
                    rhs=featB[:, gl, :],
                    start=True, stop=True,
                )
                add_dep_helper(mm.ins, join.ins, sync=True, reason="repack join")
                h1r = h1rp.tile([P, 512], F16, tag="h1r")
                if g % 4 == 3:
                    nc.vector.tensor_scalar(
                        out=h1r[:], in0=h1ps[:], scalar1=0.0, scalar2=None,
                        op0=OP.max,
                    )
                else:
                    nc.scalar.activation(out=h1r[:], in_=h1ps[:], func=AF.Relu)
                tiles[g] = h1r
            return tiles

        def edge_h1r_of(sb, j):
            if sb not in edge_h1r:
                edge_h1r.clear()
                edge_h1r[sb] = {"featT": edge_mm1(sb), "tiles": {}}
            st = edge_h1r[sb]
            if j not in st["tiles"]:
                st["tiles"].update(edge_mm1_block(st["featT"], j // 8))
            return st["tiles"][j]

        mlp_tail("e", NSB_E, CH_E * 32, eW2, eb2, eW3_16, eb3bc, e_gbc,
                 eout_r, edge_h1r_of)

        # ---------------------------------------------------------
        # NODE branch
        # ---------------------------------------------------------
        n_ngrp = (CH_N * 4) // 16
        nout_r = d_nout.ap().rearrange(
            "(sb grp tl p) f -> sb grp p tl f", sb=1, grp=n_ngrp, tl=16, p=P
        )
        velr = d_velp.ap().rearrange("(p cg) f -> p cg f", p=P)
        posr = d_posp.ap().rearrange("(p cg) f -> p cg f", p=P)

        node_h1r: dict = {}

        def node_mm1(sb):
            # f-major: featn[p, c, f, g]; transpose -> partition 4f+g
            featn = nodep.tile([P, CH_N, 32, 4], F32, tag="nfeat")
            vtile = nodep.tile([P, CH_N * 4, 10], F32, tag="nvel")
            nc.sync.dma_start(out=vtile[:], in_=velr)
            nc.vector.tensor_copy(
                out=featn[:, :, 0:10, :],
                in_=vtile[:].rearrange("p (c g) f -> p c f g", c=CH_N),
            )
            ptile = nodep.tile([P, CH_N * 4, 2], F32, tag="npos")
            nc.sync.dma_start(out=ptile[:], in_=posr)
            pv = ptile[:].rearrange("p (c g) x -> p c x g", c=CH_N)
            nc.vector.tensor_scalar(
                out=featn[:, :, 26:30:2, :], in0=pv, scalar1=0.0,
                scalar2=R_CONN, op0=OP.max, op1=OP.min,
            )
            upos = nodep.tile([P, CH_N * 4, 2], F32, tag="nupos")
            nc.vector.tensor_scalar(
                out=upos[:], in0=ptile[:], scalar1=-1.0, scalar2=1.0,
                op0=OP.mult, op1=OP.add,
            )
            nc.vector.tensor_scalar(
                out=featn[:, :, 27:31:2, :],
                in0=upos[:].rearrange("p (c g) x -> p c x g", c=CH_N),
                scalar1=0.0,
                scalar2=R_CONN, op0=OP.max, op1=OP.min,
            )
            mtile = nodep.tile([P, CH_N * 4, 16], F32, tag="nmat")
            nc.sync.dma_start(
                out=mtile[:],
                in_=d_memb.ap().rearrange("(p cg) f -> p cg f", p=P),
            )
            nc.vector.tensor_copy(
                out=featn[:, :, 10:26, :],
                in_=mtile[:].rearrange("p (c g) f -> p c f g", c=CH_N),
            )
            nc.vector.memset(featn[:, :, 30, :], 1.0)
            nc.vector.memset(featn[:, :, 31, :], 0.0)

            featTn = nodep.tile([P, CH_N, P], F16, tag="nfeatT")
            for c in range(CH_N):
                tp = tpp.tile([P, P], F32, tag="tp")
                nc.tensor.transpose(
                    out=tp[:], in_=featn[:, c, :, :], identity=ident[:]
                )
                nc.vector.tensor_copy(out=featTn[:, c, :], in_=tp[:])

            # repack: feature f lives at partitions [4f, 4f+4) (g inner);
            # move to featBn[f, g, :] so matmul rhs starts at partition 0.
            featBn = nodep.tile([31, 4, CH_N * P], F16, tag="nfeatB")
            for f in range(31):
                nc.sync.dma_start(
                    out=featBn[f:f + 1, :, :],
                    in_=featTn[4 * f:4 * f + 4, :, :],
                )
            jtn = scp.tile([31, 1], F16, tag="njoin")
            njoin = nc.vector.tensor_copy(out=jtn[:], in_=featBn[:, 0, 0:1])

            return featBn, njoin

        def node_mm1_block(featBn_join, g):
            featBn, njoin = featBn_join
            tiles = {}
            nq = CH_N // 4
            for q in range(nq):
                h1ps = h12p.tile([P, 512], F32, tag="h12")
                mm = nc.tensor.matmul(
                    out=h1ps[:],
                    lhsT=nW1aug[0:31, :],
                    rhs=featBn[:, g, 512 * q:512 * (q + 1)],
                    start=True, stop=True,
                )
                add_dep_helper(mm.ins, njoin.ins, sync=True, reason="repack join")
                h1r = h1rp.tile([P, 512], F16, tag="h1r")
                nc.scalar.activation(out=h1r[:], in_=h1ps[:], func=AF.Relu)
                tiles[g * nq + q] = h1r
            return tiles

        def node_h1r_of(sb, j):
            if sb not in node_h1r:
                node_h1r[sb] = {"featBn": node_mm1(sb), "tiles": {}}
            st = node_h1r[sb]
            if j not in st["tiles"]:
                st["tiles"].update(node_mm1_block(st["featBn"], j // 4))
            return st["tiles"][j]

        mlp_tail("n", 1, CH_N * 4, nW2, nb2, nW3_16, nb3bc, n_gbc,
                 nout_r, node_h1r_of)

    nc.compile()
    return nc


def _get_program(e_triv, n_triv):
    key = (e_triv, n_triv)
    if key not in _PROGRAM_CACHE:
        _PROGRAM_CACHE[key] = _build_program(e_triv, n_triv)
    return _PROGRAM_CACHE[key]


def _pad_table(t, rows):
    out = np.zeros((rows, t.shape[1]), np.float32)
    out[:t.shape[0]] = t
    return out


def _slot_perm_edges(arr):
    """[E_PAD, ...] -> [NSB_E, P, CH_E*32, ...]; slot (p,c,g) <- o."""
    tail = arr.shape[1:]
    a = arr.reshape(NSB_E, 32, CH_E, P, *tail).transpose(
        0, 3, 2, 1, *range(4, 4 + len(tail))
    )
    return np.ascontiguousarray(a.reshape(NSB_E, P, CH_E * 32, *tail))


def _slot_perm_nodes(arr):
    """[N_PAD,...] -> row r = p*(CH_N*4) + c*4 + g for o = g*(CH_N*128) + c*128 + p."""
    tail = arr.shape[1:]
    a = arr.reshape(4, CH_N, P, *tail).transpose(2, 1, 0, *range(3, 3 + len(tail)))
    return np.ascontiguousarray(a.reshape(N_PAD, *tail))


def kernel(
    materials, positions, velocities, neighbor_idxs,
    mat_W, mat_b, nW1, nb1, nW2, nb2, nW3, nb3, n_gamma, n_beta,
    eW1, eb1, eW2, eb2, eW3, eb3, e_gamma, e_beta,
):
    materials = np.asarray(materials)
    positions_np = np.ascontiguousarray(np.asarray(positions, np.float32)[0])
    velocities = np.asarray(velocities, np.float32)
    neighbor_idxs_np = np.asarray(neighbor_idxs)

    e_gamma = np.ascontiguousarray(np.asarray(e_gamma, np.float32))
    e_beta = np.ascontiguousarray(np.asarray(e_beta, np.float32))
    n_gamma = np.ascontiguousarray(np.asarray(n_gamma, np.float32))
    n_beta = np.ascontiguousarray(np.asarray(n_beta, np.float32))
    e_triv = bool(np.all(e_gamma == 1.0) and np.all(e_beta == 0.0))
    n_triv = bool(np.all(n_gamma == 1.0) and np.all(n_beta == 0.0))

    nc = _get_program(e_triv, n_triv)

    recv = np.asarray(neighbor_idxs_np[:, 1], np.int32)
    send = np.asarray(neighbor_idxs_np[:, 2], np.int32)

    def f32c(x, shape=None):
        a = np.asarray(x, np.float32)
        if shape is not None:
            a = a.reshape(shape)
        return np.ascontiguousarray(a)

    common = {
        "eW1": f32c(eW1), "eb1": f32c(eb1, (1, HID)),
        "eW2": f32c(eW2), "eb2": f32c(eb2, (HID, 1)),
        "eW3": f32c(eW3), "eb3": f32c(eb3, (1, HID)),
        "e_gamma": e_gamma, "e_beta": e_beta,
        "mat_b": f32c(mat_b, (16, 1)),
        "nW1": f32c(nW1), "nb1": f32c(nb1, (1, HID)),
        "nW2": f32c(nW2), "nb2": f32c(nb2, (HID, 1)),
        "nW3": f32c(nW3), "nb3": f32c(nb3, (1, HID)),
        "n_gamma": n_gamma, "n_beta": n_beta,
    }

    in_maps = []
    for c in range(N_CORES):
        e0 = c * E_CORE
        prpad = np.zeros((E_PAD, 2), np.float32)
        pspad = np.zeros((E_PAD, 2), np.float32)
        prpad[:E_CORE] = positions_np[recv[e0:e0 + E_CORE]]
        pspad[:E_CORE] = positions_np[send[e0:e0 + E_CORE]]

        n0 = c * N_CORE
        velpad = np.zeros((N_PAD, 10), np.float32)
        velpad[:N_CORE] = velocities[0, n0:n0 + N_CORE].reshape(N_CORE, 10)
        pospad = np.zeros((N_PAD, 2), np.float32)
        pospad[:N_CORE] = positions_np[n0:n0 + N_CORE]
        membpad = np.zeros((N_PAD, 16), np.float32)
        membpad[:N_CORE] = np.asarray(mat_W, np.float32)[materials[0, n0:n0 + N_CORE]]

        m = dict(common)
        m["pr_in"] = _slot_perm_edges(prpad)
        m["ps_in"] = _slot_perm_edges(pspad)
        m["velp"] = _slot_perm_nodes(velpad)
        m["posp"] = _slot_perm_nodes(pospad)
        m["memb_in"] = _slot_perm_nodes(membpad)
        in_maps.append(m)

    _ensure_ntff_hook()
    res = bass_utils.run_bass_kernel_spmd(
        nc, in_maps, core_ids=list(range(N_CORES))
    )
    if res.exec_time_ns is not None:
        print(f"HW exec time: {res.exec_time_ns} ns")

    nodes = np.empty((1, N_TOT, HID), np.float32)
    edges = np.empty((E_TOT, HID), np.float32)
    for c in range(N_CORES):
        out = res.results[c]
        edges[c * E_CORE:(c + 1) * E_CORE] = out["edges_out"][:E_CORE]
        nodes[0, c * N_CORE:(c + 1) * N_CORE] = out["nodes_out"][:N_CORE]
    return nodes, edges, neighbor_idxs_np


# revision 31
# speedup vs baseline: 1.0128x; 1.0128x over previous
"""Trainium2 Bass kernel for nn_Encoder (GNN message-passing encoder).

Self-contained: accepts FULL inputs, shards across 8 NeuronCores (data
parallel over particles/edges, params replicated), runs one SPMD Bass
program via run_bass_kernel_spmd, and gathers the full output.

Outputs match reference: (nodes [1,N,128] f32, edges [E,128] f32,
neighbor_idxs [E,3] i32 passthrough).

Pipeline (per core), all in fp32 with fp32r matmuls:
  edges: indirect-gather P[recv],P[send] -> rel/dist -> packed PE
  transpose to feature-major -> mm1(K=4, ones-row folds b1) -> relu ->
  mm2 -> relu(+b2) -> edge-major mm3 (lhsT=h2r tile, rhs=W3 zero-padded
  to 256 cols for full-rate fp32r) + fp16 ones x b3 bias matmul ->
  PSUM->SBUF evac -> batched bn_stats LayerNorm -> per-tile affine apply
  -> edge-major output DMA.  Nodes: same skeleton with K=31 features
  (vel, mat_W gather (mat_b folded into bias row on device), wall clips).
"""

import os
from contextlib import ExitStack

import numpy as np

import concourse.bacc as bacc
import concourse.bass as bass
import concourse.mybir as mybir
import concourse.tile as tile
from concourse import bass_utils
from concourse.bass import IndirectOffsetOnAxis
from concourse.masks import make_identity
from bass_rust import add_dep_helper

F32 = mybir.dt.float32
F32R = mybir.dt.float32r
F16 = mybir.dt.float16
I32 = mybir.dt.int32
AF = mybir.ActivationFunctionType
OP = mybir.AluOpType

P = 128
N_CORES = 8
E_TOT = 1_000_000
N_TOT = 50_000
HID = 128
R_CONN = 0.015
EPS = 1e-5

# ---- geometry (NSB_E env-overridable for small-scale simulator tests) ----
CH_E = 4                                    # transpose chunks / edge superblock
NSB_E = int(os.environ.get("K_NSB_E", 8))   # edge superblocks per core
CH_N = 16                                   # node chunks (of 512 nodes)

SB_E = CH_E * 32 * P                        # edges per superblock (16384)
E_PAD = NSB_E * SB_E                        # padded edges per core (131072)
SB_N = CH_N * 4 * P                         # padded nodes per core (8192)
N_PAD = SB_N

E_CORE = E_TOT // N_CORES                   # 125000
N_CORE = N_TOT // N_CORES                   # 6250

_PROGRAM_CACHE: dict = {}

def _ensure_ntff_hook():
    """Install the axon NTFF profiling hook if the glue module is absent.

    Only needed when BASS_TRACE=1; harmless otherwise."""
    try:
        import antenv.axon_hooks  # noqa: F401
        return
    except ImportError:
        pass
    import sys
    import types

    import antenv

    mod = types.ModuleType("antenv.axon_hooks")
    mod._hook = None

    def set_axon_ntff_profile_hook(h):
        mod._hook = h

    def get_axon_ntff_profile_hook():
        return mod._hook

    mod.set_axon_ntff_profile_hook = set_axon_ntff_profile_hook
    mod.get_axon_ntff_profile_hook = get_axon_ntff_profile_hook
    sys.modules["antenv.axon_hooks"] = mod
    antenv.axon_hooks = mod
    try:
        from trn_agent_boot.trn_boot import _ntff_profile_via_ctypes

        hook = _ntff_profile_via_ctypes("/opt/axon/libaxon_pjrt.so")
        if hook is not None:
            set_axon_ntff_profile_hook(hook)
    except Exception:
        pass




def _ln_combine(nc, scp, stats, nt, name, epst):
    """bn_stats even/odd 6-tuples [128,nt,6] -> (s, b) with y = s*x + b
    equivalent to (x - mean) / sqrt(var + EPS)."""
    me = stats[:, :, 1]
    mo = stats[:, :, 4]
    m2e = stats[:, :, 2]
    m2o = stats[:, :, 5]
    msum = scp.tile([P, nt], F32, tag=f"{name}_msum")
    nc.vector.tensor_tensor(out=msum[:], in0=me, in1=mo, op=OP.add)
    dlt = scp.tile([P, nt], F32, tag=f"{name}_dlt")
    nc.vector.tensor_tensor(out=dlt[:], in0=me, in1=mo, op=OP.subtract)
    dlt2 = scp.tile([P, nt], F32, tag=f"{name}_dlt2")
    nc.vector.tensor_tensor(out=dlt2[:], in0=dlt[:], in1=dlt[:], op=OP.mult)
    m2s = scp.tile([P, nt], F32, tag=f"{name}_m2s")
    nc.vector.tensor_tensor(out=m2s[:], in0=m2e, in1=m2o, op=OP.add)
    # combined M2 = m2e + m2o + (ne*no/n) * dlt^2 ; ne=no=64 -> 32
    dlt2b = scp.tile([P, nt], F32, tag=f"{name}_dlt2b")
    nc.vector.tensor_scalar(
        out=dlt2b[:], in0=dlt2[:], scalar1=float(HID // 4), scalar2=None, op0=OP.mult
    )
    m2 = scp.tile([P, nt], F32, tag=f"{name}_m2")
    nc.vector.tensor_tensor(out=m2[:], in0=m2s[:], in1=dlt2b[:], op=OP.add)
    sd = scp.tile([P, nt], F32, tag=f"{name}_sd")
    nc.scalar.activation(
        out=sd[:], in_=m2[:], func=AF.Sqrt, bias=epst[:], scale=1.0 / HID
    )
    s = scp.tile([P, nt], F32, tag=f"{name}_s")
    nc.vector.reciprocal(out=s[:], in_=sd[:])
    # b = -0.5 * msum * s
    nms = scp.tile([P, nt], F32, tag=f"{name}_nms")
    nc.vector.tensor_tensor(out=nms[:], in0=msum[:], in1=s[:], op=OP.mult)
    b = scp.tile([P, nt], F32, tag=f"{name}_b")
    nc.vector.tensor_scalar(
        out=b[:], in0=nms[:], scalar1=-0.5, scalar2=None, op0=OP.mult
    )
    return s, b


def _build_program(e_gb_trivial: bool, n_gb_trivial: bool):
    nc = bacc.Bacc("TRN2", target_bir_lowering=False, debug=False)

    # ----------------- DRAM tensors -----------------
    d_pr = nc.dram_tensor("pr_in", [NSB_E, P, CH_E * 32, 2], F32, kind="ExternalInput")
    d_ps = nc.dram_tensor("ps_in", [NSB_E, P, CH_E * 32, 2], F32, kind="ExternalInput")
    d_eW1 = nc.dram_tensor("eW1", [3, HID], F32, kind="ExternalInput")
    d_eb1 = nc.dram_tensor("eb1", [1, HID], F32, kind="ExternalInput")
    d_eW2 = nc.dram_tensor("eW2", [HID, HID], F32, kind="ExternalInput")
    d_eb2 = nc.dram_tensor("eb2", [HID, 1], F32, kind="ExternalInput")
    d_eW3 = nc.dram_tensor("eW3", [HID, HID], F32, kind="ExternalInput")
    d_eb3 = nc.dram_tensor("eb3", [1, HID], F32, kind="ExternalInput")
    d_eg = nc.dram_tensor("e_gamma", [HID], F32, kind="ExternalInput")
    d_ebt = nc.dram_tensor("e_beta", [HID], F32, kind="ExternalInput")

    d_velp = nc.dram_tensor("velp", [N_PAD, 10], F32, kind="ExternalInput")
    d_posp = nc.dram_tensor("posp", [N_PAD, 2], F32, kind="ExternalInput")
    d_memb = nc.dram_tensor("memb_in", [N_PAD, 16], F32, kind="ExternalInput")
    d_matb = nc.dram_tensor("mat_b", [16, 1], F32, kind="ExternalInput")
    d_nW1 = nc.dram_tensor("nW1", [30, HID], F32, kind="ExternalInput")
    d_nb1 = nc.dram_tensor("nb1", [1, HID], F32, kind="ExternalInput")
    d_nW2 = nc.dram_tensor("nW2", [HID, HID], F32, kind="ExternalInput")
    d_nb2 = nc.dram_tensor("nb2", [HID, 1], F32, kind="ExternalInput")
    d_nW3 = nc.dram_tensor("nW3", [HID, HID], F32, kind="ExternalInput")
    d_nb3 = nc.dram_tensor("nb3", [1, HID], F32, kind="ExternalInput")
    d_ng = nc.dram_tensor("n_gamma", [HID], F32, kind="ExternalInput")
    d_nbt = nc.dram_tensor("n_beta", [HID], F32, kind="ExternalInput")

    d_eout = nc.dram_tensor("edges_out", [E_PAD, HID], F32, kind="ExternalOutput")
    d_nout = nc.dram_tensor("nodes_out", [N_PAD, HID], F32, kind="ExternalOutput")

    with ExitStack() as ctx:
        tc = ctx.enter_context(tile.TileContext(nc))
        const = ctx.enter_context(tc.tile_pool(name="const", bufs=1))
        idxp = ctx.enter_context(tc.tile_pool(name="idxp", bufs=3))
        gath = ctx.enter_context(tc.tile_pool(name="gath", bufs=2))
        featp = ctx.enter_context(tc.tile_pool(name="featp", bufs=2))
        sqp = ctx.enter_context(tc.tile_pool(name="sqp", bufs=2))
        ftp = ctx.enter_context(tc.tile_pool(name="ftp", bufs=2))
        fbp = ctx.enter_context(tc.tile_pool(name="fbp", bufs=2))
        nodep = ctx.enter_context(tc.tile_pool(name="nodep", bufs=1))
        h1rp = ctx.enter_context(tc.tile_pool(name="h1rp", bufs=8))
        h2rp = ctx.enter_context(tc.tile_pool(name="h2rp", bufs=3))
        zstp = ctx.enter_context(tc.tile_pool(name="zstp", bufs=3))
        st2p = ctx.enter_context(tc.tile_pool(name="st2p", bufs=2))
        statp = ctx.enter_context(tc.tile_pool(name="statp", bufs=3))
        scp = ctx.enter_context(tc.tile_pool(name="scp", bufs=3))

        tpp = ctx.enter_context(tc.tile_pool(name="tpp", bufs=2, space="PSUM"))
        h12p = ctx.enter_context(tc.tile_pool(name="h12p", bufs=2, space="PSUM"))
        m3pp = ctx.enter_context(tc.tile_pool(name="m3pp", bufs=2, space="PSUM"))

        # ----------------- constants -----------------
        ident = const.tile([P, P], F32)
        make_identity(nc, ident[:])

        ones16 = const.tile([1, HID], F16)
        nc.vector.memset(ones16[:], 1.0)

        epst = const.tile([P, 1], F32)
        nc.vector.memset(epst[:], EPS)

        def load_w(name, dram, shape, dtype=F32):
            t = const.tile(list(shape), dtype, tag=name)
            if dtype == F16:
                nc.gpsimd.dma_start(out=t[:], in_=dram.ap())  # SWDGE casts
            else:
                nc.sync.dma_start(out=t[:], in_=dram.ap())
            return t

        eW1aug = const.tile([4, HID], F16)
        nc.gpsimd.dma_start(out=eW1aug[0:3, :], in_=d_eW1.ap())
        nc.gpsimd.dma_start(out=eW1aug[3:4, :], in_=d_eb1.ap())
        eW2 = load_w("eW2", d_eW2, [HID, HID], F16)
        eb2 = load_w("eb2", d_eb2, [HID, 1])
        eW3_16 = load_w("eW3_16", d_eW3, [HID, HID], F16)
        # b3 broadcast tile (added during PSUM evac)
        eb3bc = const.tile([P, HID], F32, tag="eb3bc")
        nc.sync.dma_start(
            out=eb3bc[:], in_=bass.AP(tensor=d_eb3, offset=0, ap=[[0, P], [1, HID]])
        )

        nW2 = load_w("nW2", d_nW2, [HID, HID], F16)
        nb2 = load_w("nb2", d_nb2, [HID, 1])
        nW3_16 = load_w("nW3_16", d_nW3, [HID, HID], F16)
        nb3bc = const.tile([P, HID], F32, tag="nb3bc")
        nc.sync.dma_start(
            out=nb3bc[:], in_=bass.AP(tensor=d_nb3, offset=0, ap=[[0, P], [1, HID]])
        )

        matb = load_w("matb", d_matb, [16, 1])

        # nW1aug: rows 0..29 = nW1, row 30 = nb1 + mat_b @ nW1[10:26], row 31 = 0
        nW1aug = const.tile([32, HID], F16)
        nc.gpsimd.dma_start(out=nW1aug[0:30, :], in_=d_nW1.ap())
        nb1tmp = load_w("nb1tmp", d_nb1, [1, HID])
        # base-partition-0 copy of nW1 rows 10..25 (matmul rhs must start at 0)
        nW1mid = const.tile([16, HID], F32)
        nc.sync.dma_start(out=nW1mid[:], in_=d_nW1.ap()[10:26, :])
        nb1x = tpp.tile([1, HID], F32, tag="tp")
        nc.tensor.matmul(
            out=nb1x[:], lhsT=matb[:], rhs=nW1mid[:], start=True, stop=True
        )
        nb1row = const.tile([1, HID], F16)
        nc.vector.tensor_tensor(
            out=nb1row[:], in0=nb1tmp[:], in1=nb1x[:], op=OP.add
        )
        nc.sync.dma_start(out=nW1aug[30:31, :], in_=nb1row[:])

        def gb_tiles(d_g, d_b, name):
            g_bc = const.tile([P, HID], F32, tag=f"{name}_gbc")
            b_bc = const.tile([P, HID], F32, tag=f"{name}_bbc")
            nc.sync.dma_start(
                out=g_bc[:], in_=bass.AP(tensor=d_g, offset=0, ap=[[0, P], [1, HID]])
            )
            nc.sync.dma_start(
                out=b_bc[:], in_=bass.AP(tensor=d_b, offset=0, ap=[[0, P], [1, HID]])
            )
            return g_bc, b_bc

        e_gbc = None if e_gb_trivial else gb_tiles(d_eg, d_ebt, "e")
        n_gbc = None if n_gb_trivial else gb_tiles(d_ng, d_nbt, "n")

        def bc16(t):  # [P, HID] tile -> [P, 16, HID] free-broadcast AP
            a = t[:]
            return bass.AP(tensor=a.tensor, offset=a.offset,
                           ap=[a.ap[0], [0, 16], a.ap[1]])

        # ---------------------------------------------------------
        # shared tail: mm2 -> relu2 -> mm3(+bias) -> evac -> LN -> out
        # ---------------------------------------------------------
        def mlp_tail(pfx, n_sb, sb_tiles, W2, b2, W3_16, b3bc, gbc,
                     out_dram_r, h1r_of):
            ngrp = sb_tiles // 16
            for sb in range(n_sb):
                for grp in range(ngrp):
                    zst = zstp.tile([P, 8, 2 * HID], F32, tag="zst")
                    stats = statp.tile([P, 8, 8], F32, tag=f"{pfx}_stats")
                    for half in range(2):
                        zp = m3pp.tile([P, 8, HID], F32, tag="zp")
                        for sub, jj in enumerate(
                            (grp * 4 + half * 2, grp * 4 + half * 2 + 1)
                        ):
                            h1r = h1r_of(sb, jj)
                            h2ps = h12p.tile([P, 512], F32, tag="h12")
                            nc.tensor.matmul(
                                out=h2ps[:],
                                lhsT=W2[:],
                                rhs=h1r[:],
                                start=True, stop=True,
                            )
                            h2r = h2rp.tile([P, 512], F16, tag="h2r")
                            nc.scalar.activation(
                                out=h2r[:], in_=h2ps[:], func=AF.Relu,
                                bias=b2[:], scale=1.0,
                            )
                            for tl in range(4):
                                nc.tensor.matmul(
                                    out=zp[:, sub * 4 + tl, :],
                                    lhsT=h2r[:, tl * P:(tl + 1) * P],
                                    rhs=W3_16[:],
                                    start=True, stop=True,
                                )
                        b3v = b3bc[:]
                        z = zst[:, half * 4:(half + 1) * 4, :]
                        zi = bass.AP(
                            tensor=z.tensor, offset=z.offset,
                            ap=[z.ap[0], z.ap[1], [1, 2], [2, HID]],
                        )
                        nc.vector.tensor_tensor(
                            out=zi,
                            in0=zp[:, :, :].rearrange("p (pr tl) f -> p pr tl f", tl=2),
                            in1=bass.AP(tensor=b3v.tensor, offset=b3v.offset,
                                        ap=[b3v.ap[0], [0, 4], [0, 2], b3v.ap[1]]),
                            op=OP.add,
                        )
                    for pr in range(8):
                        nc.vector.bn_stats(
                            out=stats[:, pr, 0:6],
                            in_=zst[:, pr, :],
                        )
                    # per-tile mean/var live directly in even/odd slots
                    m_v = stats[:, :, 1:5:3]       # [P, 8, 2] means
                    var_v = stats[:, :, 2:6:3]     # [P, 8, 2] M2 (count=128)
                    sd = scp.tile([P, 8, 2], F32, tag=f"{pfx}_sd")
                    nc.scalar.activation(
                        out=sd[:], in_=var_v, func=AF.Sqrt, bias=epst[:],
                        scale=1.0 / HID,
                    )
                    s_t = scp.tile([P, 8, 2], F32, tag=f"{pfx}_s")
                    nc.vector.reciprocal(out=s_t[:], in_=sd[:])
                    nms = scp.tile([P, 8, 2], F32, tag=f"{pfx}_nms")
                    nc.vector.tensor_tensor(out=nms[:], in0=m_v, in1=s_t[:], op=OP.mult)
                    b_t = scp.tile([P, 8, 2], F32, tag=f"{pfx}_b")
                    nc.vector.tensor_scalar(
                        out=b_t[:], in0=nms[:], scalar1=-1.0, scalar2=None, op0=OP.mult
                    )
                    st2 = st2p.tile([P, 16, HID], F32, tag="st2")
                    for tl in range(16):
                        pr, par = tl // 2, tl % 2
                        sc = s_t[:, pr, par:par + 1]
                        bi = b_t[:, pr, par:par + 1]
                        zin = zst[:, pr, par:par + 2 * HID - 1:2]
                        if tl % 4 == 0:
                            nc.scalar.activation(
                                out=st2[:, tl, :], in_=zin,
                                func=AF.Identity, bias=bi, scale=sc,
                            )
                        else:
                            nc.gpsimd.tensor_scalar(
                                out=st2[:, tl, :], in0=zin,
                                scalar1=sc, scalar2=bi, op0=OP.mult, op1=OP.add,
                            )
                    if gbc is not None:
                        g_bc, b_bc = gbc
                        nc.vector.tensor_tensor(
                            out=st2[:], in0=st2[:], in1=bc16(g_bc), op=OP.mult
                        )
                        nc.vector.tensor_tensor(
                            out=st2[:], in0=st2[:], in1=bc16(b_bc), op=OP.add
                        )
                    nc.sync.dma_start(out=out_dram_r[sb, grp], in_=st2[:])

        # ---------------------------------------------------------
        # EDGE branch
        # ---------------------------------------------------------
        e_ngrp = (CH_E * 32) // 16  # staging groups per edge superblock
        eout_r = d_eout.ap().rearrange(
            "(sb grp tl p) f -> sb grp p tl f", sb=NSB_E, grp=e_ngrp, tl=16, p=P
        )

        edge_h1r: dict = {}

        def edge_mm1(sb):
            pr = gath.tile([P, CH_E * 32, 2], F32, tag="pr")
            nc.sync.dma_start(out=pr[:], in_=d_pr.ap()[sb])
            ps = gath.tile([P, CH_E * 32, 2], F32, tag="ps")
            nc.sync.dma_start(out=ps[:], in_=d_ps.ap()[sb])
            # f-major packed layout: feat[p, c, f, g] -> transpose puts
            # feature f of group g at partition 32f+g (contiguous f bands)
            feat = featp.tile([P, CH_E, 4, 32], F32, tag="efeat")
            prv = pr[:].rearrange("p (c g) x -> p c x g", c=CH_E)
            psv = ps[:].rearrange("p (c g) x -> p c x g", c=CH_E)
            nc.gpsimd.tensor_tensor(
                out=feat[:, :, 0:2, :], in0=prv, in1=psv, op=OP.subtract
            )
            sq = sqp.tile([P, CH_E, 2, 32], F32, tag="esq")
            nc.gpsimd.tensor_tensor(
                out=sq[:], in0=feat[:, :, 0:2, :], in1=feat[:, :, 0:2, :], op=OP.mult
            )
            d2 = sqp.tile([P, CH_E, 32], F32, tag="ed2")
            nc.gpsimd.tensor_tensor(
                out=d2[:], in0=sq[:, :, 0, :], in1=sq[:, :, 1, :], op=OP.add
            )
            nc.scalar.activation(out=feat[:, :, 2, :], in_=d2[:], func=AF.Sqrt)
            nc.vector.memset(feat[:, :, 3, :], 1.0)

            featT = ftp.tile([P, CH_E, P], F16, tag="efeatT")
            for c in range(CH_E):
                tp = tpp.tile([P, P], F32, tag="tp")
                nc.tensor.transpose(out=tp[:], in_=feat[:, c, :, :], identity=ident[:])
                nc.vector.tensor_copy(out=featT[:, c, :], in_=tp[:])

            return featT

        def edge_mm1_block(featT, blk):
            # repack so each group's 4 feature rows start at partition 0
            # (matmul operands must have base partition 0/32/64)
            featB = fbp.tile([4, 8, CH_E * P], F16, tag="efeatB")
            for f in range(4):
                s = 32 * f + 8 * blk
                nc.sync.dma_start(
                    out=featB[f:f + 1, :, :],
                    in_=featT[s:s + 8, :, :],
                )
            # join op: single wait target covering all 4 repack DMAs so the
            # consuming matmuls stay under the HW sync-wait-command limit
            jt = scp.tile([4, 1], F16, tag="ejoin")
            join = nc.vector.tensor_copy(out=jt[:], in_=featB[:, 0, 0:1])
            tiles = {}
            for gl in range(8):
                g = blk * 8 + gl
                h1ps = h12p.tile([P, 512], F32, tag="h12")
                mm = nc.tensor.matmul(
                    out=h1ps[:],
                    lhsT=eW1aug[:],
                    rhs=featB[:, gl, :],
                    start=True, stop=True,
                )
                add_dep_helper(mm.ins, join.ins, sync=True, reason="repack join")
                h1r = h1rp.tile([P, 512], F16, tag="h1r")
                nc.scalar.activation(out=h1r[:], in_=h1ps[:], func=AF.Relu)
                tiles[g] = h1r
            return tiles

        def edge_h1r_of(sb, j):
            if sb not in edge_h1r:
                edge_h1r.clear()
                edge_h1r[sb] = {"featT": edge_mm1(sb), "tiles": {}}
            st = edge_h1r[sb]
            if j not in st["tiles"]:
                st["tiles"].update(edge_mm1_block(st["featT"], j // 8))
            return st["tiles"][j]

        mlp_tail("e", NSB_E, CH_E * 32, eW2, eb2, eW3_16, eb3bc, e_gbc,
                 eout_r, edge_h1r_of)

        # ---------------------------------------------------------
        # NODE branch
        # ---------------------------------------------------------
        n_ngrp = (CH_N * 4) // 16
        nout_r = d_nout.ap().rearrange(
            "(sb grp tl p) f -> sb grp p tl f", sb=1, grp=n_ngrp, tl=16, p=P
        )
        velr = d_velp.ap().rearrange("(p cg) f -> p cg f", p=P)
        posr = d_posp.ap().rearrange("(p cg) f -> p cg f", p=P)

        node_h1r: dict = {}

        def node_mm1(sb):
            # f-major: featn[p, c, f, g]; transpose -> partition 4f+g
            featn = nodep.tile([P, CH_N, 32, 4], F32, tag="nfeat")
            vtile = nodep.tile([P, CH_N * 4, 10], F32, tag="nvel")
            nc.sync.dma_start(out=vtile[:], in_=velr)
            nc.vector.tensor_copy(
                out=featn[:, :, 0:10, :],
                in_=vtile[:].rearrange("p (c g) f -> p c f g", c=CH_N),
            )
            ptile = nodep.tile([P, CH_N * 4, 2], F32, tag="npos")
            nc.sync.dma_start(out=ptile[:], in_=posr)
            pv = ptile[:].rearrange("p (c g) x -> p c x g", c=CH_N)
            nc.vector.tensor_scalar(
                out=featn[:, :, 26:30:2, :], in0=pv, scalar1=0.0,
                scalar2=R_CONN, op0=OP.max, op1=OP.min,
            )
            upos = nodep.tile([P, CH_N * 4, 2], F32, tag="nupos")
            nc.vector.tensor_scalar(
                out=upos[:], in0=ptile[:], scalar1=-1.0, scalar2=1.0,
                op0=OP.mult, op1=OP.add,
            )
            nc.vector.tensor_scalar(
                out=featn[:, :, 27:31:2, :],
                in0=upos[:].rearrange("p (c g) x -> p c x g", c=CH_N),
                scalar1=0.0,
                scalar2=R_CONN, op0=OP.max, op1=OP.min,
            )
            mtile = nodep.tile([P, CH_N * 4, 16], F32, tag="nmat")
            nc.sync.dma_start(
                out=mtile[:],
                in_=d_memb.ap().rearrange("(p cg) f -> p cg f", p=P),
            )
            nc.vector.tensor_copy(
                out=featn[:, :, 10:26, :],
                in_=mtile[:].rearrange("p (c g) f -> p c f g", c=CH_N),
            )
            nc.vector.memset(featn[:, :, 30, :], 1.0)
            nc.vector.memset(featn[:, :, 31, :], 0.0)

            featTn = nodep.tile([P, CH_N, P], F16, tag="nfeatT")
            for c in range(CH_N):
                tp = tpp.tile([P, P], F32, tag="tp")
                nc.tensor.transpose(
                    out=tp[:], in_=featn[:, c, :, :], identity=ident[:]
                )
                nc.vector.tensor_copy(out=featTn[:, c, :], in_=tp[:])

            # repack: feature f lives at partitions [4f, 4f+4) (g inner);
            # move to featBn[f, g, :] so matmul rhs starts at partition 0.
            featBn = nodep.tile([31, 4, CH_N * P], F16, tag="nfeatB")
            for f in range(31):
                nc.sync.dma_start(
                    out=featBn[f:f + 1, :, :],
                    in_=featTn[4 * f:4 * f + 4, :, :],
                )
            jtn = scp.tile([31, 1], F16, tag="njoin")
            njoin = nc.vector.tensor_copy(out=jtn[:], in_=featBn[:, 0, 0:1])

            return featBn, njoin

        def node_mm1_block(featBn_join, g):
            featBn, njoin = featBn_join
            tiles = {}
            nq = CH_N // 4
            for q in range(nq):
                h1ps = h12p.tile([P, 512], F32, tag="h12")
                mm = nc.tensor.matmul(
                    out=h1ps[:],
                    lhsT=nW1aug[0:31, :],
                    rhs=featBn[:, g, 512 * q:512 * (q + 1)],
                    start=True, stop=True,
                )
                add_dep_helper(mm.ins, njoin.ins, sync=True, reason="repack join")
                h1r = h1rp.tile([P, 512], F16, tag="h1r")
                nc.scalar.activation(out=h1r[:], in_=h1ps[:], func=AF.Relu)
                tiles[g * nq + q] = h1r
            return tiles

        def node_h1r_of(sb, j):
            if sb not in node_h1r:
                node_h1r[sb] = {"featBn": node_mm1(sb), "tiles": {}}
            st = node_h1r[sb]
            if j not in st["tiles"]:
                st["tiles"].update(node_mm1_block(st["featBn"], j // 4))
            return st["tiles"][j]

        mlp_tail("n", 1, CH_N * 4, nW2, nb2, nW3_16, nb3bc, n_gbc,
                 nout_r, node_h1r_of)

    nc.compile()
    return nc


def _get_program(e_triv, n_triv):
    key = (e_triv, n_triv)
    if key not in _PROGRAM_CACHE:
        _PROGRAM_CACHE[key] = _build_program(e_triv, n_triv)
    return _PROGRAM_CACHE[key]


def _pad_table(t, rows):
    out = np.zeros((rows, t.shape[1]), np.float32)
    out[:t.shape[0]] = t
    return out


def _slot_perm_edges(arr):
    """[E_PAD, ...] -> [NSB_E, P, CH_E*32, ...]; slot (p,c,g) <- o."""
    tail = arr.shape[1:]
    a = arr.reshape(NSB_E, 32, CH_E, P, *tail).transpose(
        0, 3, 2, 1, *range(4, 4 + len(tail))
    )
    return np.ascontiguousarray(a.reshape(NSB_E, P, CH_E * 32, *tail))


def _slot_perm_nodes(arr):
    """[N_PAD,...] -> row r = p*(CH_N*4) + c*4 + g for o = g*(CH_N*128) + c*128 + p."""
    tail = arr.shape[1:]
    a = arr.reshape(4, CH_N, P, *tail).transpose(2, 1, 0, *range(3, 3 + len(tail)))
    return np.ascontiguousarray(a.reshape(N_PAD, *tail))


def kernel(
    materials, positions, velocities, neighbor_idxs,
    mat_W, mat_b, nW1, nb1, nW2, nb2, nW3, nb3, n_gamma, n_beta,
    eW1, eb1, eW2, eb2, eW3, eb3, e_gamma, e_beta,
):
    materials = np.asarray(materials)
    positions_np = np.ascontiguousarray(np.asarray(positions, np.float32)[0])
    velocities = np.asarray(velocities, np.float32)
    neighbor_idxs_np = np.asarray(neighbor_idxs)

    e_gamma = np.ascontiguousarray(np.asarray(e_gamma, np.float32))
    e_beta = np.ascontiguousarray(np.asarray(e_beta, np.float32))
    n_gamma = np.ascontiguousarray(np.asarray(n_gamma, np.float32))
    n_beta = np.ascontiguousarray(np.asarray(n_beta, np.float32))
    e_triv = bool(np.all(e_gamma == 1.0) and np.all(e_beta == 0.0))
    n_triv = bool(np.all(n_gamma == 1.0) and np.all(n_beta == 0.0))

    nc = _get_program(e_triv, n_triv)

    recv = np.asarray(neighbor_idxs_np[:, 1], np.int32)
    send = np.asarray(neighbor_idxs_np[:, 2], np.int32)

    def f32c(x, shape=None):
        a = np.asarray(x, np.float32)
        if shape is not None:
            a = a.reshape(shape)
        return np.ascontiguousarray(a)

    common = {
        "eW1": f32c(eW1), "eb1": f32c(eb1, (1, HID)),
        "eW2": f32c(eW2), "eb2": f32c(eb2, (HID, 1)),
        "eW3": f32c(eW3), "eb3": f32c(eb3, (1, HID)),
        "e_gamma": e_gamma, "e_beta": e_beta,
        "mat_b": f32c(mat_b, (16, 1)),
        "nW1": f32c(nW1), "nb1": f32c(nb1, (1, HID)),
        "nW2": f32c(nW2), "nb2": f32c(nb2, (HID, 1)),
        "nW3": f32c(nW3), "nb3": f32c(nb3, (1, HID)),
        "n_gamma": n_gamma, "n_beta": n_beta,
    }

    in_maps = []
    for c in range(N_CORES):
        e0 = c * E_CORE
        prpad = np.zeros((E_PAD, 2), np.float32)
        pspad = np.zeros((E_PAD, 2), np.float32)
        prpad[:E_CORE] = positions_np[recv[e0:e0 + E_CORE]]
        pspad[:E_CORE] = positions_np[send[e0:e0 + E_CORE]]

        n0 = c * N_CORE
        velpad = np.zeros((N_PAD, 10), np.float32)
        velpad[:N_CORE] = velocities[0, n0:n0 + N_CORE].reshape(N_CORE, 10)
        pospad = np.zeros((N_PAD, 2), np.float32)
        pospad[:N_CORE] = positions_np[n0:n0 + N_CORE]
        membpad = np.zeros((N_PAD, 16), np.float32)
        membpad[:N_CORE] = np.asarray(mat_W, np.float32)[materials[0, n0:n0 + N_CORE]]

        m = dict(common)
        m["pr_in"] = _slot_perm_edges(prpad)
        m["ps_in"] = _slot_perm_edges(pspad)
        m["velp"] = _slot_perm_nodes(velpad)
        m["posp"] = _slot_perm_nodes(pospad)
        m["memb_in"] = _slot_perm_nodes(membpad)
        in_maps.append(m)

    _ensure_ntff_hook()
    res = bass_utils.run_bass_kernel_spmd(
        nc, in_maps, core_ids=list(range(N_CORES))
    )
    if res.exec_time_ns is not None:
        print(f"HW exec time: {res.exec_time_ns} ns")

    nodes = np.empty((1, N_TOT, HID), np.float32)
    edges = np.empty((E_TOT, HID), np.float32)
    for c in range(N_CORES):
        out = res.results[c]
        edges[c * E_CORE:(c + 1) * E_CORE] = out["edges_out"][:E_CORE]
        nodes[0, c * N_CORE:(c + 1) * N_CORE] = out["nodes_out"][:N_CORE]
    return nodes, edges, neighbor_idxs_np


# revision 32
# speedup vs baseline: 1.0130x; 1.0002x over previous
"""Trainium2 Bass kernel for nn_Encoder (GNN message-passing encoder).

Self-contained: accepts FULL inputs, shards across 8 NeuronCores (data
parallel over particles/edges, params replicated), runs one SPMD Bass
program via run_bass_kernel_spmd, and gathers the full output.

Outputs match reference: (nodes [1,N,128] f32, edges [E,128] f32,
neighbor_idxs [E,3] i32 passthrough).

Pipeline (per core), all in fp32 with fp32r matmuls:
  edges: indirect-gather P[recv],P[send] -> rel/dist -> packed PE
  transpose to feature-major -> mm1(K=4, ones-row folds b1) -> relu ->
  mm2 -> relu(+b2) -> edge-major mm3 (lhsT=h2r tile, rhs=W3 zero-padded
  to 256 cols for full-rate fp32r) + fp16 ones x b3 bias matmul ->
  PSUM->SBUF evac -> batched bn_stats LayerNorm -> per-tile affine apply
  -> edge-major output DMA.  Nodes: same skeleton with K=31 features
  (vel, mat_W gather (mat_b folded into bias row on device), wall clips).
"""

import os
from contextlib import ExitStack

import numpy as np

import concourse.bacc as bacc
import concourse.bass as bass
import concourse.mybir as mybir
import concourse.tile as tile
from concourse import bass_utils
from concourse.bass import IndirectOffsetOnAxis
from concourse.masks import make_identity
from bass_rust import add_dep_helper

F32 = mybir.dt.float32
F32R = mybir.dt.float32r
F16 = mybir.dt.float16
I32 = mybir.dt.int32
AF = mybir.ActivationFunctionType
OP = mybir.AluOpType

P = 128
N_CORES = 8
E_TOT = 1_000_000
N_TOT = 50_000
HID = 128
R_CONN = 0.015
EPS = 1e-5

# ---- geometry (NSB_E env-overridable for small-scale simulator tests) ----
CH_E = 4                                    # transpose chunks / edge superblock
NSB_E = int(os.environ.get("K_NSB_E", 8))   # edge superblocks per core
CH_N = 16                                   # node chunks (of 512 nodes)

SB_E = CH_E * 32 * P                        # edges per superblock (16384)
E_PAD = NSB_E * SB_E                        # padded edges per core (131072)
SB_N = CH_N * 4 * P                         # padded nodes per core (8192)
N_PAD = SB_N

E_CORE = E_TOT // N_CORES                   # 125000
N_CORE = N_TOT // N_CORES                   # 6250

_PROGRAM_CACHE: dict = {}

def _ensure_ntff_hook():
    """Install the axon NTFF profiling hook if the glue module is absent.

    Only needed when BASS_TRACE=1; harmless otherwise."""
    try:
        import antenv.axon_hooks  # noqa: F401
        return
    except ImportError:
        pass
    import sys
    import types

    import antenv

    mod = types.ModuleType("antenv.axon_hooks")
    mod._hook = None

    def set_axon_ntff_profile_hook(h):
        mod._hook = h

    def get_axon_ntff_profile_hook():
        return mod._hook

    mod.set_axon_ntff_profile_hook = set_axon_ntff_profile_hook
    mod.get_axon_ntff_profile_hook = get_axon_ntff_profile_hook
    sys.modules["antenv.axon_hooks"] = mod
    antenv.axon_hooks = mod
    try:
        from trn_agent_boot.trn_boot import _ntff_profile_via_ctypes

        hook = _ntff_profile_via_ctypes("/opt/axon/libaxon_pjrt.so")
        if hook is not None:
            set_axon_ntff_profile_hook(hook)
    except Exception:
        pass




def _ln_combine(nc, scp, stats, nt, name, epst):
    """bn_stats even/odd 6-tuples [128,nt,6] -> (s, b) with y = s*x + b
    equivalent to (x - mean) / sqrt(var + EPS)."""
    me = stats[:, :, 1]
    mo = stats[:, :, 4]
    m2e = stats[:, :, 2]
    m2o = stats[:, :, 5]
    msum = scp.tile([P, nt], F32, tag=f"{name}_msum")
    nc.vector.tensor_tensor(out=msum[:], in0=me, in1=mo, op=OP.add)
    dlt = scp.tile([P, nt], F32, tag=f"{name}_dlt")
    nc.vector.tensor_tensor(out=dlt[:], in0=me, in1=mo, op=OP.subtract)
    dlt2 = scp.tile([P, nt], F32, tag=f"{name}_dlt2")
    nc.vector.tensor_tensor(out=dlt2[:], in0=dlt[:], in1=dlt[:], op=OP.mult)
    m2s = scp.tile([P, nt], F32, tag=f"{name}_m2s")
    nc.vector.tensor_tensor(out=m2s[:], in0=m2e, in1=m2o, op=OP.add)
    # combined M2 = m2e + m2o + (ne*no/n) * dlt^2 ; ne=no=64 -> 32
    dlt2b = scp.tile([P, nt], F32, tag=f"{name}_dlt2b")
    nc.vector.tensor_scalar(
        out=dlt2b[:], in0=dlt2[:], scalar1=float(HID // 4), scalar2=None, op0=OP.mult
    )
    m2 = scp.tile([P, nt], F32, tag=f"{name}_m2")
    nc.vector.tensor_tensor(out=m2[:], in0=m2s[:], in1=dlt2b[:], op=OP.add)
    sd = scp.tile([P, nt], F32, tag=f"{name}_sd")
    nc.scalar.activation(
        out=sd[:], in_=m2[:], func=AF.Sqrt, bias=epst[:], scale=1.0 / HID
    )
    s = scp.tile([P, nt], F32, tag=f"{name}_s")
    nc.vector.reciprocal(out=s[:], in_=sd[:])
    # b = -0.5 * msum * s
    nms = scp.tile([P, nt], F32, tag=f"{name}_nms")
    nc.vector.tensor_tensor(out=nms[:], in0=msum[:], in1=s[:], op=OP.mult)
    b = scp.tile([P, nt], F32, tag=f"{name}_b")
    nc.vector.tensor_scalar(
        out=b[:], in0=nms[:], scalar1=-0.5, scalar2=None, op0=OP.mult
    )
    return s, b


def _build_program(e_gb_trivial: bool, n_gb_trivial: bool):
    nc = bacc.Bacc("TRN2", target_bir_lowering=False, debug=False)

    # ----------------- DRAM tensors -----------------
    d_pr = nc.dram_tensor("pr_in", [NSB_E, P, CH_E * 32, 2], F32, kind="ExternalInput")
    d_ps = nc.dram_tensor("ps_in", [NSB_E, P, CH_E * 32, 2], F32, kind="ExternalInput")
    d_eW1 = nc.dram_tensor("eW1", [3, HID], F32, kind="ExternalInput")
    d_eb1 = nc.dram_tensor("eb1", [1, HID], F32, kind="ExternalInput")
    d_eW2 = nc.dram_tensor("eW2", [HID, HID], F32, kind="ExternalInput")
    d_eb2 = nc.dram_tensor("eb2", [HID, 1], F32, kind="ExternalInput")
    d_eW3 = nc.dram_tensor("eW3", [HID, HID], F32, kind="ExternalInput")
    d_eb3 = nc.dram_tensor("eb3", [1, HID], F32, kind="ExternalInput")
    d_eg = nc.dram_tensor("e_gamma", [HID], F32, kind="ExternalInput")
    d_ebt = nc.dram_tensor("e_beta", [HID], F32, kind="ExternalInput")

    d_velp = nc.dram_tensor("velp", [N_PAD, 10], F32, kind="ExternalInput")
    d_posp = nc.dram_tensor("posp", [N_PAD, 2], F32, kind="ExternalInput")
    d_memb = nc.dram_tensor("memb_in", [N_PAD, 16], F32, kind="ExternalInput")
    d_matb = nc.dram_tensor("mat_b", [16, 1], F32, kind="ExternalInput")
    d_nW1 = nc.dram_tensor("nW1", [30, HID], F32, kind="ExternalInput")
    d_nb1 = nc.dram_tensor("nb1", [1, HID], F32, kind="ExternalInput")
    d_nW2 = nc.dram_tensor("nW2", [HID, HID], F32, kind="ExternalInput")
    d_nb2 = nc.dram_tensor("nb2", [HID, 1], F32, kind="ExternalInput")
    d_nW3 = nc.dram_tensor("nW3", [HID, HID], F32, kind="ExternalInput")
    d_nb3 = nc.dram_tensor("nb3", [1, HID], F32, kind="ExternalInput")
    d_ng = nc.dram_tensor("n_gamma", [HID], F32, kind="ExternalInput")
    d_nbt = nc.dram_tensor("n_beta", [HID], F32, kind="ExternalInput")

    d_eout = nc.dram_tensor("edges_out", [E_PAD, HID], F32, kind="ExternalOutput")
    d_nout = nc.dram_tensor("nodes_out", [N_PAD, HID], F32, kind="ExternalOutput")

    with ExitStack() as ctx:
        tc = ctx.enter_context(tile.TileContext(nc))
        const = ctx.enter_context(tc.tile_pool(name="const", bufs=1))
        idxp = ctx.enter_context(tc.tile_pool(name="idxp", bufs=3))
        gath = ctx.enter_context(tc.tile_pool(name="gath", bufs=3))
        featp = ctx.enter_context(tc.tile_pool(name="featp", bufs=3))
        sqp = ctx.enter_context(tc.tile_pool(name="sqp", bufs=2))
        ftp = ctx.enter_context(tc.tile_pool(name="ftp", bufs=3))
        fbp = ctx.enter_context(tc.tile_pool(name="fbp", bufs=3))
        nodep = ctx.enter_context(tc.tile_pool(name="nodep", bufs=1))
        h1rp = ctx.enter_context(tc.tile_pool(name="h1rp", bufs=10))
        h2rp = ctx.enter_context(tc.tile_pool(name="h2rp", bufs=5))
        zstp = ctx.enter_context(tc.tile_pool(name="zstp", bufs=4))
        st2p = ctx.enter_context(tc.tile_pool(name="st2p", bufs=4))
        statp = ctx.enter_context(tc.tile_pool(name="statp", bufs=4))
        scp = ctx.enter_context(tc.tile_pool(name="scp", bufs=4))

        tpp = ctx.enter_context(tc.tile_pool(name="tpp", bufs=2, space="PSUM"))
        h12p = ctx.enter_context(tc.tile_pool(name="h12p", bufs=2, space="PSUM"))
        m3pp = ctx.enter_context(tc.tile_pool(name="m3pp", bufs=2, space="PSUM"))

        # ----------------- constants -----------------
        ident = const.tile([P, P], F32)
        make_identity(nc, ident[:])

        ones16 = const.tile([1, HID], F16)
        nc.vector.memset(ones16[:], 1.0)

        epst = const.tile([P, 1], F32)
        nc.vector.memset(epst[:], EPS)

        def load_w(name, dram, shape, dtype=F32):
            t = const.tile(list(shape), dtype, tag=name)
            if dtype == F16:
                nc.gpsimd.dma_start(out=t[:], in_=dram.ap())  # SWDGE casts
            else:
                nc.sync.dma_start(out=t[:], in_=dram.ap())
            return t

        eW1aug = const.tile([4, HID], F16)
        nc.gpsimd.dma_start(out=eW1aug[0:3, :], in_=d_eW1.ap())
        nc.gpsimd.dma_start(out=eW1aug[3:4, :], in_=d_eb1.ap())
        eW2 = load_w("eW2", d_eW2, [HID, HID], F16)
        eb2 = load_w("eb2", d_eb2, [HID, 1])
        eW3_16 = load_w("eW3_16", d_eW3, [HID, HID], F16)
        # b3 broadcast tile (added during PSUM evac)
        eb3bc = const.tile([P, HID], F32, tag="eb3bc")
        nc.sync.dma_start(
            out=eb3bc[:], in_=bass.AP(tensor=d_eb3, offset=0, ap=[[0, P], [1, HID]])
        )

        nW2 = load_w("nW2", d_nW2, [HID, HID], F16)
        nb2 = load_w("nb2", d_nb2, [HID, 1])
        nW3_16 = load_w("nW3_16", d_nW3, [HID, HID], F16)
        nb3bc = const.tile([P, HID], F32, tag="nb3bc")
        nc.sync.dma_start(
            out=nb3bc[:], in_=bass.AP(tensor=d_nb3, offset=0, ap=[[0, P], [1, HID]])
        )

        matb = load_w("matb", d_matb, [16, 1])

        # nW1aug: rows 0..29 = nW1, row 30 = nb1 + mat_b @ nW1[10:26], row 31 = 0
        nW1aug = const.tile([32, HID], F16)
        nc.gpsimd.dma_start(out=nW1aug[0:30, :], in_=d_nW1.ap())
        nb1tmp = load_w("nb1tmp", d_nb1, [1, HID])
        # base-partition-0 copy of nW1 rows 10..25 (matmul rhs must start at 0)
        nW1mid = const.tile([16, HID], F32)
        nc.sync.dma_start(out=nW1mid[:], in_=d_nW1.ap()[10:26, :])
        nb1x = tpp.tile([1, HID], F32, tag="tp")
        nc.tensor.matmul(
            out=nb1x[:], lhsT=matb[:], rhs=nW1mid[:], start=True, stop=True
        )
        nb1row = const.tile([1, HID], F16)
        nc.vector.tensor_tensor(
            out=nb1row[:], in0=nb1tmp[:], in1=nb1x[:], op=OP.add
        )
        nc.sync.dma_start(out=nW1aug[30:31, :], in_=nb1row[:])

        def gb_tiles(d_g, d_b, name):
            g_bc = const.tile([P, HID], F32, tag=f"{name}_gbc")
            b_bc = const.tile([P, HID], F32, tag=f"{name}_bbc")
            nc.sync.dma_start(
                out=g_bc[:], in_=bass.AP(tensor=d_g, offset=0, ap=[[0, P], [1, HID]])
            )
            nc.sync.dma_start(
                out=b_bc[:], in_=bass.AP(tensor=d_b, offset=0, ap=[[0, P], [1, HID]])
            )
            return g_bc, b_bc

        e_gbc = None if e_gb_trivial else gb_tiles(d_eg, d_ebt, "e")
        n_gbc = None if n_gb_trivial else gb_tiles(d_ng, d_nbt, "n")

        def bc16(t):  # [P, HID] tile -> [P, 16, HID] free-broadcast AP
            a = t[:]
            return bass.AP(tensor=a.tensor, offset=a.offset,
                           ap=[a.ap[0], [0, 16], a.ap[1]])

        # ---------------------------------------------------------
        # shared tail: mm2 -> relu2 -> mm3(+bias) -> evac -> LN -> out
        # ---------------------------------------------------------
        def mlp_tail(pfx, n_sb, sb_tiles, W2, b2, W3_16, b3bc, gbc,
                     out_dram_r, h1r_of):
            ngrp = sb_tiles // 16
            for sb in range(n_sb):
                for grp in range(ngrp):
                    zst = zstp.tile([P, 8, 2 * HID], F32, tag="zst")
                    stats = statp.tile([P, 8, 8], F32, tag=f"{pfx}_stats")
                    for half in range(2):
                        zp = m3pp.tile([P, 8, HID], F32, tag="zp")
                        for sub, jj in enumerate(
                            (grp * 4 + half * 2, grp * 4 + half * 2 + 1)
                        ):
                            h1r = h1r_of(sb, jj)
                            h2ps = h12p.tile([P, 512], F32, tag="h12")
                            nc.tensor.matmul(
                                out=h2ps[:],
                                lhsT=W2[:],
                                rhs=h1r[:],
                                start=True, stop=True,
                            )
                            h2r = h2rp.tile([P, 512], F16, tag="h2r")
                            nc.scalar.activation(
                                out=h2r[:], in_=h2ps[:], func=AF.Relu,
                                bias=b2[:], scale=1.0,
                            )
                            for tl in range(4):
                                nc.tensor.matmul(
                                    out=zp[:, sub * 4 + tl, :],
                                    lhsT=h2r[:, tl * P:(tl + 1) * P],
                                    rhs=W3_16[:],
                                    start=True, stop=True,
                                )
                        b3v = b3bc[:]
                        z = zst[:, half * 4:(half + 1) * 4, :]
                        zi = bass.AP(
                            tensor=z.tensor, offset=z.offset,
                            ap=[z.ap[0], z.ap[1], [1, 2], [2, HID]],
                        )
                        nc.vector.tensor_tensor(
                            out=zi,
                            in0=zp[:, :, :].rearrange("p (pr tl) f -> p pr tl f", tl=2),
                            in1=bass.AP(tensor=b3v.tensor, offset=b3v.offset,
                                        ap=[b3v.ap[0], [0, 4], [0, 2], b3v.ap[1]]),
                            op=OP.add,
                        )
                    for pr in range(8):
                        nc.vector.bn_stats(
                            out=stats[:, pr, 0:6],
                            in_=zst[:, pr, :],
                        )
                    # per-tile mean/var live directly in even/odd slots
                    m_v = stats[:, :, 1:5:3]       # [P, 8, 2] means
                    var_v = stats[:, :, 2:6:3]     # [P, 8, 2] M2 (count=128)
                    sd = scp.tile([P, 8, 2], F32, tag=f"{pfx}_sd")
                    nc.scalar.activation(
                        out=sd[:], in_=var_v, func=AF.Sqrt, bias=epst[:],
                        scale=1.0 / HID,
                    )
                    s_t = scp.tile([P, 8, 2], F32, tag=f"{pfx}_s")
                    nc.vector.reciprocal(out=s_t[:], in_=sd[:])
                    nms = scp.tile([P, 8, 2], F32, tag=f"{pfx}_nms")
                    nc.vector.tensor_tensor(out=nms[:], in0=m_v, in1=s_t[:], op=OP.mult)
                    b_t = scp.tile([P, 8, 2], F32, tag=f"{pfx}_b")
                    nc.vector.tensor_scalar(
                        out=b_t[:], in0=nms[:], scalar1=-1.0, scalar2=None, op0=OP.mult
                    )
                    st2 = st2p.tile([P, 16, HID], F32, tag="st2")
                    for tl in range(16):
                        pr, par = tl // 2, tl % 2
                        sc = s_t[:, pr, par:par + 1]
                        bi = b_t[:, pr, par:par + 1]
                        zin = zst[:, pr, par:par + 2 * HID - 1:2]
                        if tl % 4 == 0:
                            nc.scalar.activation(
                                out=st2[:, tl, :], in_=zin,
                                func=AF.Identity, bias=bi, scale=sc,
                            )
                        else:
                            nc.gpsimd.tensor_scalar(
                                out=st2[:, tl, :], in0=zin,
                                scalar1=sc, scalar2=bi, op0=OP.mult, op1=OP.add,
                            )
                    if gbc is not None:
                        g_bc, b_bc = gbc
                        nc.vector.tensor_tensor(
                            out=st2[:], in0=st2[:], in1=bc16(g_bc), op=OP.mult
                        )
                        nc.vector.tensor_tensor(
                            out=st2[:], in0=st2[:], in1=bc16(b_bc), op=OP.add
                        )
                    nc.sync.dma_start(out=out_dram_r[sb, grp], in_=st2[:])

        # ---------------------------------------------------------
        # EDGE branch
        # ---------------------------------------------------------
        e_ngrp = (CH_E * 32) // 16  # staging groups per edge superblock
        eout_r = d_eout.ap().rearrange(
            "(sb grp tl p) f -> sb grp p tl f", sb=NSB_E, grp=e_ngrp, tl=16, p=P
        )

        edge_h1r: dict = {}

        def edge_mm1(sb):
            pr = gath.tile([P, CH_E * 32, 2], F32, tag="pr")
            nc.sync.dma_start(out=pr[:], in_=d_pr.ap()[sb])
            ps = gath.tile([P, CH_E * 32, 2], F32, tag="ps")
            nc.sync.dma_start(out=ps[:], in_=d_ps.ap()[sb])
            # f-major packed layout: feat[p, c, f, g] -> transpose puts
            # feature f of group g at partition 32f+g (contiguous f bands)
            feat = featp.tile([P, CH_E, 4, 32], F32, tag="efeat")
            prv = pr[:].rearrange("p (c g) x -> p c x g", c=CH_E)
            psv = ps[:].rearrange("p (c g) x -> p c x g", c=CH_E)
            nc.gpsimd.tensor_tensor(
                out=feat[:, :, 0:2, :], in0=prv, in1=psv, op=OP.subtract
            )
            sq = sqp.tile([P, CH_E, 2, 32], F32, tag="esq")
            nc.gpsimd.tensor_tensor(
                out=sq[:], in0=feat[:, :, 0:2, :], in1=feat[:, :, 0:2, :], op=OP.mult
            )
            d2 = sqp.tile([P, CH_E, 32], F32, tag="ed2")
            nc.gpsimd.tensor_tensor(
                out=d2[:], in0=sq[:, :, 0, :], in1=sq[:, :, 1, :], op=OP.add
            )
            nc.scalar.activation(out=feat[:, :, 2, :], in_=d2[:], func=AF.Sqrt)
            nc.vector.memset(feat[:, :, 3, :], 1.0)

            featT = ftp.tile([P, CH_E, P], F16, tag="efeatT")
            for c in range(CH_E):
                tp = tpp.tile([P, P], F32, tag="tp")
                nc.tensor.transpose(out=tp[:], in_=feat[:, c, :, :], identity=ident[:])
                nc.vector.tensor_copy(out=featT[:, c, :], in_=tp[:])

            return featT

        def edge_mm1_block(featT, blk):
            # repack so each group's 4 feature rows start at partition 0
            # (matmul operands must have base partition 0/32/64)
            featB = fbp.tile([4, 8, CH_E * P], F16, tag="efeatB")
            for f in range(4):
                s = 32 * f + 8 * blk
                nc.sync.dma_start(
                    out=featB[f:f + 1, :, :],
                    in_=featT[s:s + 8, :, :],
                )
            # join op: single wait target covering all 4 repack DMAs so the
            # consuming matmuls stay under the HW sync-wait-command limit
            jt = scp.tile([4, 1], F16, tag="ejoin")
            join = nc.vector.tensor_copy(out=jt[:], in_=featB[:, 0, 0:1])
            tiles = {}
            for gl in range(8):
                g = blk * 8 + gl
                h1ps = h12p.tile([P, 512], F32, tag="h12")
                mm = nc.tensor.matmul(
                    out=h1ps[:],
                    lhsT=eW1aug[:],
                    rhs=featB[:, gl, :],
                    start=True, stop=True,
                )
                add_dep_helper(mm.ins, join.ins, sync=True, reason="repack join")
                h1r = h1rp.tile([P, 512], F16, tag="h1r")
                nc.scalar.activation(out=h1r[:], in_=h1ps[:], func=AF.Relu)
                tiles[g] = h1r
            return tiles

        def edge_h1r_of(sb, j):
            if sb not in edge_h1r:
                edge_h1r.clear()
                edge_h1r[sb] = {"featT": edge_mm1(sb), "tiles": {}}
            st = edge_h1r[sb]
            if j not in st["tiles"]:
                st["tiles"].update(edge_mm1_block(st["featT"], j // 8))
            return st["tiles"][j]

        mlp_tail("e", NSB_E, CH_E * 32, eW2, eb2, eW3_16, eb3bc, e_gbc,
                 eout_r, edge_h1r_of)

        # ---------------------------------------------------------
        # NODE branch
        # ---------------------------------------------------------
        n_ngrp = (CH_N * 4) // 16
        nout_r = d_nout.ap().rearrange(
            "(sb grp tl p) f -> sb grp p tl f", sb=1, grp=n_ngrp, tl=16, p=P
        )
        velr = d_velp.ap().rearrange("(p cg) f -> p cg f", p=P)
        posr = d_posp.ap().rearrange("(p cg) f -> p cg f", p=P)

        node_h1r: dict = {}

        def node_mm1(sb):
            # f-major: featn[p, c, f, g]; transpose -> partition 4f+g
            featn = nodep.tile([P, CH_N, 32, 4], F32, tag="nfeat")
            vtile = nodep.tile([P, CH_N * 4, 10], F32, tag="nvel")
            nc.sync.dma_start(out=vtile[:], in_=velr)
            nc.vector.tensor_copy(
                out=featn[:, :, 0:10, :],
                in_=vtile[:].rearrange("p (c g) f -> p c f g", c=CH_N),
            )
            ptile = nodep.tile([P, CH_N * 4, 2], F32, tag="npos")
            nc.sync.dma_start(out=ptile[:], in_=posr)
            pv = ptile[:].rearrange("p (c g) x -> p c x g", c=CH_N)
            nc.vector.tensor_scalar(
                out=featn[:, :, 26:30:2, :], in0=pv, scalar1=0.0,
                scalar2=R_CONN, op0=OP.max, op1=OP.min,
            )
            upos = nodep.tile([P, CH_N * 4, 2], F32, tag="nupos")
            nc.vector.tensor_scalar(
                out=upos[:], in0=ptile[:], scalar1=-1.0, scalar2=1.0,
                op0=OP.mult, op1=OP.add,
            )
            nc.vector.tensor_scalar(
                out=featn[:, :, 27:31:2, :],
                in0=upos[:].rearrange("p (c g) x -> p c x g", c=CH_N),
                scalar1=0.0,
                scalar2=R_CONN, op0=OP.max, op1=OP.min,
            )
            mtile = nodep.tile([P, CH_N * 4, 16], F32, tag="nmat")
            nc.sync.dma_start(
                out=mtile[:],
                in_=d_memb.ap().rearrange("(p cg) f -> p cg f", p=P),
            )
            nc.vector.tensor_copy(
                out=featn[:, :, 10:26, :],
                in_=mtile[:].rearrange("p (c g) f -> p c f g", c=CH_N),
            )
            nc.vector.memset(featn[:, :, 30, :], 1.0)
            nc.vector.memset(featn[:, :, 31, :], 0.0)

            featTn = nodep.tile([P, CH_N, P], F16, tag="nfeatT")
            for c in range(CH_N):
                tp = tpp.tile([P, P], F32, tag="tp")
                nc.tensor.transpose(
                    out=tp[:], in_=featn[:, c, :, :], identity=ident[:]
                )
                nc.vector.tensor_copy(out=featTn[:, c, :], in_=tp[:])

            # repack: feature f lives at partitions [4f, 4f+4) (g inner);
            # move to featBn[f, g, :] so matmul rhs starts at partition 0.
            featBn = nodep.tile([31, 4, CH_N * P], F16, tag="nfeatB")
            for f in range(31):
                nc.sync.dma_start(
                    out=featBn[f:f + 1, :, :],
                    in_=featTn[4 * f:4 * f + 4, :, :],
                )
            jtn = scp.tile([31, 1], F16, tag="njoin")
            njoin = nc.vector.tensor_copy(out=jtn[:], in_=featBn[:, 0, 0:1])

            return featBn, njoin

        def node_mm1_block(featBn_join, g):
            featBn, njoin = featBn_join
            tiles = {}
            nq = CH_N // 4
            for q in range(nq):
                h1ps = h12p.tile([P, 512], F32, tag="h12")
                mm = nc.tensor.matmul(
                    out=h1ps[:],
                    lhsT=nW1aug[0:31, :],
                    rhs=featBn[:, g, 512 * q:512 * (q + 1)],
                    start=True, stop=True,
                )
                add_dep_helper(mm.ins, njoin.ins, sync=True, reason="repack join")
                h1r = h1rp.tile([P, 512], F16, tag="h1r")
                nc.scalar.activation(out=h1r[:], in_=h1ps[:], func=AF.Relu)
                tiles[g * nq + q] = h1r
            return tiles

        def node_h1r_of(sb, j):
            if sb not in node_h1r:
                node_h1r[sb] = {"featBn": node_mm1(sb), "tiles": {}}
            st = node_h1r[sb]
            if j not in st["tiles"]:
                st["tiles"].update(node_mm1_block(st["featBn"], j // 4))
            return st["tiles"][j]

        mlp_tail("n", 1, CH_N * 4, nW2, nb2, nW3_16, nb3bc, n_gbc,
                 nout_r, node_h1r_of)

    nc.compile()
    return nc


def _get_program(e_triv, n_triv):
    key = (e_triv, n_triv)
    if key not in _PROGRAM_CACHE:
        _PROGRAM_CACHE[key] = _build_program(e_triv, n_triv)
    return _PROGRAM_CACHE[key]


def _pad_table(t, rows):
    out = np.zeros((rows, t.shape[1]), np.float32)
    out[:t.shape[0]] = t
    return out


def _slot_perm_edges(arr):
    """[E_PAD, ...] -> [NSB_E, P, CH_E*32, ...]; slot (p,c,g) <- o."""
    tail = arr.shape[1:]
    a = arr.reshape(NSB_E, 32, CH_E, P, *tail).transpose(
        0, 3, 2, 1, *range(4, 4 + len(tail))
    )
    return np.ascontiguousarray(a.reshape(NSB_E, P, CH_E * 32, *tail))


def _slot_perm_nodes(arr):
    """[N_PAD,...] -> row r = p*(CH_N*4) + c*4 + g for o = g*(CH_N*128) + c*128 + p."""
    tail = arr.shape[1:]
    a = arr.reshape(4, CH_N, P, *tail).transpose(2, 1, 0, *range(3, 3 + len(tail)))
    return np.ascontiguousarray(a.reshape(N_PAD, *tail))


def kernel(
    materials, positions, velocities, neighbor_idxs,
    mat_W, mat_b, nW1, nb1, nW2, nb2, nW3, nb3, n_gamma, n_beta,
    eW1, eb1, eW2, eb2, eW3, eb3, e_gamma, e_beta,
):
    materials = np.asarray(materials)
    positions_np = np.ascontiguousarray(np.asarray(positions, np.float32)[0])
    velocities = np.asarray(velocities, np.float32)
    neighbor_idxs_np = np.asarray(neighbor_idxs)

    e_gamma = np.ascontiguousarray(np.asarray(e_gamma, np.float32))
    e_beta = np.ascontiguousarray(np.asarray(e_beta, np.float32))
    n_gamma = np.ascontiguousarray(np.asarray(n_gamma, np.float32))
    n_beta = np.ascontiguousarray(np.asarray(n_beta, np.float32))
    e_triv = bool(np.all(e_gamma == 1.0) and np.all(e_beta == 0.0))
    n_triv = bool(np.all(n_gamma == 1.0) and np.all(n_beta == 0.0))

    nc = _get_program(e_triv, n_triv)

    recv = np.asarray(neighbor_idxs_np[:, 1], np.int32)
    send = np.asarray(neighbor_idxs_np[:, 2], np.int32)

    def f32c(x, shape=None):
        a = np.asarray(x, np.float32)
        if shape is not None:
            a = a.reshape(shape)
        return np.ascontiguousarray(a)

    common = {
        "eW1": f32c(eW1), "eb1": f32c(eb1, (1, HID)),
        "eW2": f32c(eW2), "eb2": f32c(eb2, (HID, 1)),
        "eW3": f32c(eW3), "eb3": f32c(eb3, (1, HID)),
        "e_gamma": e_gamma, "e_beta": e_beta,
        "mat_b": f32c(mat_b, (16, 1)),
        "nW1": f32c(nW1), "nb1": f32c(nb1, (1, HID)),
        "nW2": f32c(nW2), "nb2": f32c(nb2, (HID, 1)),
        "nW3": f32c(nW3), "nb3": f32c(nb3, (1, HID)),
        "n_gamma": n_gamma, "n_beta": n_beta,
    }

    in_maps = []
    for c in range(N_CORES):
        e0 = c * E_CORE
        prpad = np.zeros((E_PAD, 2), np.float32)
        pspad = np.zeros((E_PAD, 2), np.float32)
        prpad[:E_CORE] = positions_np[recv[e0:e0 + E_CORE]]
        pspad[:E_CORE] = positions_np[send[e0:e0 + E_CORE]]

        n0 = c * N_CORE
        velpad = np.zeros((N_PAD, 10), np.float32)
        velpad[:N_CORE] = velocities[0, n0:n0 + N_CORE].reshape(N_CORE, 10)
        pospad = np.zeros((N_PAD, 2), np.float32)
        pospad[:N_CORE] = positions_np[n0:n0 + N_CORE]
        membpad = np.zeros((N_PAD, 16), np.float32)
        membpad[:N_CORE] = np.asarray(mat_W, np.float32)[materials[0, n0:n0 + N_CORE]]

        m = dict(common)
        m["pr_in"] = _slot_perm_edges(prpad)
        m["ps_in"] = _slot_perm_edges(pspad)
        m["velp"] = _slot_perm_nodes(velpad)
        m["posp"] = _slot_perm_nodes(pospad)
        m["memb_in"] = _slot_perm_nodes(membpad)
        in_maps.append(m)

    _ensure_ntff_hook()
    res = bass_utils.run_bass_kernel_spmd(
        nc, in_maps, core_ids=list(range(N_CORES))
    )
    if res.exec_time_ns is not None:
        print(f"HW exec time: {res.exec_time_ns} ns")

    nodes = np.empty((1, N_TOT, HID), np.float32)
    edges = np.empty((E_TOT, HID), np.float32)
    for c in range(N_CORES):
        out = res.results[c]
        edges[c * E_CORE:(c + 1) * E_CORE] = out["edges_out"][:E_CORE]
        nodes[0, c * N_CORE:(c + 1) * N_CORE] = out["nodes_out"][:N_CORE]
    return nodes, edges, neighbor_idxs_np
